# revision 25
# baseline (speedup 1.0000x reference)
"""AttentionDecoder Trainium2 kernel (v2).

Strategy (8 NeuronCores, zero collectives):
  - The LSTM recurrence (T=150 serial steps) is run REDUNDANTLY with the full
    B=64 batch on every core: PE streaming time is independent of the matmul
    M dim, so full-batch costs the same as 1/8-batch, and we avoid all
    per-step communication.
  - Each core's batch ORDER is rotated (core k leads with rows 8k..8k+8), so
    the phase-3 attention/logits work (which depends only on the stored h2
    sequence) is batch-sharded 8 ways using identical SPMD code.
  - x-projection is folded into a precomputed table P = relu(E) @ W_ih.T + b
    (only 1000 vocab rows), gathered per-step via dma_gather.
  - All matmuls in bf16 (1 cyc/row vs 4 for fp32); cell state c kept in f32.

v2 phase-2 redesign (vs the v1 baseline):
  - FOLDED gate layout: each 512-wide gate is computed as [128, 256] via a
    column-tiled matmul pair (tile_position (.,0) / (.,64)), so ACT/DVE use
    all 128 lanes (2x fewer cycles) and only TWO PE transposes per step are
    needed to rebuild hT (each transpose yields two h-chunks).
  - Per-gate PSUM banks + eager per-gate activations in order [i,g,f,o]:
    the c-path (m1/m2/c2/tanh) overlaps the f/o matmul groups, so the
    serial tail is just sigmoid(o) -> h2 -> transpose -> copy.
  - No filler matmuls; the dense 40-MM burst per step keeps HAM warm.
"""

import os
import sys

import numpy as np

sys.path.insert(0, "/opt/trn_rl_repo")

import ml_dtypes  # noqa: E402

import concourse.bass as bass  # noqa: E402
import concourse.bacc as bacc  # noqa: E402
import concourse.mybir as mybir  # noqa: E402
import concourse.tile as tile  # noqa: E402
from concourse.bass_utils import run_bass_kernel_spmd  # noqa: E402
from concourse.masks import make_identity  # noqa: E402


def _install_ntff_hook():
    """The agent image's antenv lacks axon_hooks; shim it so trace=True can
    reach the ctypes NTFF profiling hook in trn_agent_boot."""
    import types
    if "antenv.axon_hooks" in sys.modules:
        return
    mod = types.ModuleType("antenv.axon_hooks")
    state = {"hook": None}
    mod.set_axon_ntff_profile_hook = lambda h: state.__setitem__("hook", h)
    mod.get_axon_ntff_profile_hook = lambda: state["hook"]
    sys.modules["antenv.axon_hooks"] = mod
    try:
        import antenv
        antenv.axon_hooks = mod
    except ImportError:
        pass
    try:
        from trn_agent_boot.trn_boot import _ntff_profile_via_ctypes
        mod.set_axon_ntff_profile_hook(
            _ntff_profile_via_ctypes("/opt/axon/libaxon_pjrt.so"))
    except Exception:
        pass


_install_ntff_hook()

BF16 = mybir.dt.bfloat16
F32 = mybir.dt.float32
NPBF16 = ml_dtypes.bfloat16

B, T, L, H, E, V = 64, 150, 256, 512, 512, 1000
G = 4 * H  # 2048
NCORES = 8
BLOC = B // NCORES  # 8
SIG = mybir.ActivationFunctionType.Sigmoid
TANH = mybir.ActivationFunctionType.Tanh
EXP = mybir.ActivationFunctionType.Exp
LN = mybir.ActivationFunctionType.Ln
RELU = mybir.ActivationFunctionType.Relu
ADD = mybir.AluOpType.add
SUB = mybir.AluOpType.subtract
MAXOP = mybir.AluOpType.max
X = mybir.AxisListType.X

N_GATHER = T * B // 128  # 75 gathers of 128 rows (2 steps each)


def build_program():
    nc = bacc.Bacc(
        "TRN2",
        target_bir_lowering=False,
        debug=False,
        num_swdge_queues=4,
    )

    # ---- I/O -----------------------------------------------------------
    whhT_d = nc.dram_tensor("whhT", [4, 128, G], BF16, kind="ExternalInput")
    wihT_d = nc.dram_tensor("wihT", [4, 128, G], BF16, kind="ExternalInput")
    embT_d = nc.dram_tensor("embT", [4, 128, V], BF16, kind="ExternalInput")
    bias2_d = nc.dram_tensor("bias2", [2, G], BF16, kind="ExternalInput")
    whmT_d = nc.dram_tensor("whmT", [4, 128, H], BF16, kind="ExternalInput")
    womT_d = nc.dram_tensor("womT", [8, 128, H], BF16, kind="ExternalInput")
    wlogT_d = nc.dram_tensor("wlogT", [4, 128, V], BF16, kind="ExternalInput")
    blog_d = nc.dram_tensor("blog", [1, V], BF16, kind="ExternalInput")
    cnn_d = nc.dram_tensor("cnn", [BLOC, 2, 128, H], BF16, kind="ExternalInput")
    cnnT_d = nc.dram_tensor("cnnT", [BLOC, 4, 128, L], BF16, kind="ExternalInput")
    idx_d = nc.dram_tensor("idx", [128, T * B // 16], mybir.dt.int16,
                           kind="ExternalInput")
    out_d = nc.dram_tensor("out", [BLOC, T, V], F32, kind="ExternalOutput")

    with tile.TileContext(nc) as tc:
        _body(nc, tc, whhT_d, wihT_d, embT_d, bias2_d, whmT_d, womT_d,
              wlogT_d, blog_d, cnn_d, cnnT_d, idx_d, out_d)
    nc.compile()
    return nc


def _copy(nc, j, out, in_):
    # psum -> sbuf copies: only DVE/ACT may read PSUM
    if j % 2 == 1:
        nc.scalar.copy(out, in_)
    else:
        nc.vector.tensor_copy(out, in_)


def _body(nc, tc, whhT_d, wihT_d, embT_d, bias2_d, whmT_d, womT_d,
          wlogT_d, blog_d, cnn_d, cnnT_d, idx_d, out_d):
    singles = tc.alloc_tile_pool(name="singles", bufs=1)

    # ---- resident constants -------------------------------------------
    whhT = singles.tile([128, 4, G], BF16)
    idx_sb = singles.tile([128, T * B // 16], mybir.dt.int16)
    nc.sync.dma_start(out=idx_sb, in_=idx_d.ap())
    ident = singles.tile([128, 128], BF16)
    make_identity(nc, ident)
    ones2 = singles.tile([2, 128], BF16)
    nc.vector.memset(ones2, 1.0)
    bias2 = singles.tile([2, G], BF16)
    nc.sync.dma_start(out=bias2, in_=bias2_d.ap())
    junkin = singles.tile([128, 512], BF16)
    nc.vector.memset(junkin, 0.0)

    # phase-3 resident weights / data (tiles allocated here; their DMAs are
    # deferred until after the gather kickoff so they don't gate phase 0)
    whmT = singles.tile([128, 4, H], BF16)
    womT = singles.tile([128, 8, H], BF16)
    wlogT = singles.tile([128, 4, V], BF16)
    blog = singles.tile([1, V], BF16)
    cnn_sb = singles.tile([128, BLOC, 2, H], BF16)   # [l%128, b, l//128, h]
    cnnT_sb = singles.tile([128, BLOC, 4, L], BF16)  # [h%128, b, h//128, l]

    # HT_sb collects transposed h2 for this core's 8 batch rows:
    # [h%128, h//128, b*T + t]
    HT = singles.tile([128, 4, BLOC * T], BF16)

    # ---- phase 0: P = relu(embT).T @ W_ih.T + (b_ih + b_hh) ------------
    drampool = tc.alloc_tile_pool(name="dram", bufs=1, space="DRAM")
    P_dram = drampool.tile([V, G], BF16)
    with tc.tile_pool(name="p0", bufs=1) as p0pool, \
         tc.tile_pool(name="p0psum", bufs=2, space="PSUM") as p0psum, \
         tc.tile_pool(name="p0junk", bufs=1, space="PSUM") as p0junk, \
         tc.tile_pool(name="p0out", bufs=3) as p0out:
        wihT = p0pool.tile([128, 4, G], BF16)
        embT = p0pool.tile([128, 4, V], BF16)
        for k in range(4):
            nc.sync.dma_start(out=wihT[:, k, :], in_=wihT_d.ap()[k])
            nc.sync.dma_start(out=embT[:, k, :], in_=embT_d.ap()[k])
            nc.scalar.activation(embT[:, k, :], embT[:, k, :], RELU)
        # whhT is not needed until the recurrence starts; keep it off the
        # phase-0 critical DMA path
        for k in range(4):
            nc.sync.dma_start(out=whhT[:, k, :], in_=whhT_d.ap()[k])
        # warm-up matmuls: keep the PE (and its HAM clock gate) busy while
        # the embT/wihT DMAs land, so phase 0 runs at 2.4 GHz
        junkp = p0junk.tile([128, 512], F32)
        for _ in range(36):
            nc.tensor.matmul(junkp[0:64, :], ident[:, 0:64], junkin,
                             start=True, stop=True)
        jout = p0out.tile([1, 8], F32, tag="jout")
        nc.vector.tensor_copy(jout, junkp[0:1, 0:8])
        junk_dram = drampool.tile([1, 8], F32)
        nc.sync.dma_start(out=junk_dram[:], in_=jout)
        for m in range(8):  # vocab chunks: 7*128 + 104
            mr = min(128, V - m * 128)
            pt = p0out.tile([128, G], BF16)
            for half in range(2):  # 2-bank psum groups so bufs=2 fits
                ps = p0psum.tile([128, 2, 512], F32)
                for k in range(4):
                    for j in range(2):
                        n = half * 2 + j
                        nc.tensor.matmul(
                            ps[:mr, j, :],
                            embT[:, k, m * 128:m * 128 + mr],
                            wihT[:, k, n * 512:(n + 1) * 512],
                            start=(k == 0), stop=False)
                for j in range(2):
                    n = half * 2 + j
                    nc.tensor.matmul(
                        ps[:mr, j, :], ones2[:, :mr],
                        bias2[:, n * 512:(n + 1) * 512],
                        start=False, stop=True)
                for j in range(2):
                    n = half * 2 + j
                    _copy(nc, n, pt[:mr, n * 512:(n + 1) * 512], ps[:mr, j, :])
            nc.sync.dma_start(out=P_dram[m * 128:m * 128 + mr, :], in_=pt[:mr, :])

    # ---- phase 2: the recurrence (folded [128, 256] gate layout) -------
    # Logical [64, 512] per-gate tensors are stored folded as [128, 256]:
    # FX[p, c] = X[p % 64, (p // 64) * 256 + c].  All elementwise ops are
    # layout-agnostic; the PE transpose of a [128, 128] slice of folded h2
    # yields TWO h-major hT chunks at once.
    with tc.tile_pool(name="gx", bufs=6) as gxpool, \
         tc.tile_pool(name="state", bufs=2) as statepool, \
         tc.tile_pool(name="work", bufs=3) as workpool, \
         tc.tile_pool(name="gpsum", bufs=1, space="PSUM") as gpsum, \
         tc.tile_pool(name="trpsum", bufs=1, space="PSUM") as trpsum:

        gx_tiles = [None] * N_GATHER
        n128 = nc.gpsimd.to_reg(128)

        def emit_gather(g):
            gxt = gxpool.tile([128, 2048], BF16, tag="gx")
            nc.gpsimd.dma_gather(
                out_ap=gxt.rearrange("p (o f) -> p o f", o=1),
                in_ap=P_dram[:],
                idxs_ap=idx_sb[:, g * 8:(g + 1) * 8],
                num_idxs=128,
                num_idxs_reg=n128,
                elem_size=G,
                queue_num=g % 4,
            )
            gx_tiles[g] = gxt

        # first 6 gathers up front; the rest are emitted inside the step
        # loop so the gpsimd FIFO interleaves them with the stash copies
        for g in range(6):
            emit_gather(g)

        # deferred phase-3 constant DMAs: they arrive during the recurrence
        for k in range(4):
            nc.sync.dma_start(out=whmT[:, k, :], in_=whmT_d.ap()[k])
            nc.sync.dma_start(out=wlogT[:, k, :], in_=wlogT_d.ap()[k])
        for k in range(8):
            nc.sync.dma_start(out=womT[:, k, :], in_=womT_d.ap()[k])
        nc.sync.dma_start(out=blog, in_=blog_d.ap())
        for b in range(BLOC):
            for k in range(2):
                nc.sync.dma_start(out=cnn_sb[:, b, k, :], in_=cnn_d.ap()[b, k])
            for k in range(4):
                nc.sync.dma_start(out=cnnT_sb[:, b, k, :], in_=cnnT_d.ap()[b, k])

        # h state lives TRANSPOSED (h-major) as [128, 2(A/B), 2(sub), 64]:
        # h-chunk k is h2T[:, k % 2, k // 2, :].  c stays batch-folded f32.
        h2T = statepool.tile([128, 2, 2, 64], BF16, tag="hT")
        nc.vector.memset(h2T, 0.0)
        cF = statepool.tile([128, 256], F32, tag="cF")
        nc.vector.memset(cF, 0.0)

        def emit_folds(t):
            """Fold the gathered x-projection for step t into fresh gate
            banks (start=True).  No hT dependency, so these run during the
            previous step's tail."""
            po = (t % 2) * 64
            gx = gx_tiles[t // 2]
            idg = ident[po:po + 64, po:po + 64]
            gps = []
            for n in range(4):
                gp = gpsum.tile([128, 256], F32, tag=f"g{n}")
                c0 = n * 512
                nc.tensor.matmul(gp[0:64, :], idg,
                                 gx[po:po + 64, c0:c0 + 256],
                                 start=True, stop=False, tile_position=(po, 0))
                nc.tensor.matmul(gp[64:128, :], idg,
                                 gx[po:po + 64, c0 + 256:c0 + 512],
                                 start=True, stop=False, tile_position=(po, 64))
                gps.append(gp)
            return gps

        # Per-gate W_hh pair order: gates complete early->late as
        # [i, g, f, o] (for the eager ACT chain), while k in {0,2} pairs
        # lead so the A-half of the previous step's h2T unblocks the burst
        # ~0.4us before the B-half is ready.
        PAIR_ORDER = [(0, 0), (0, 2), (1, 0), (1, 2), (0, 1), (0, 3),
                      (1, 1), (1, 3), (2, 0), (2, 2), (3, 0), (3, 2),
                      (2, 1), (2, 3), (3, 1), (3, 3)]

        junk2 = trpsum.tile([128, 256], F32, tag="junk2")
        gps = emit_folds(0)
        for t in range(T):
            # Gate order along G: [i | g | f | o] (see perm in _prep_inputs).
            # Eager per-gate ACT lets the c-path overlap the f/o MM groups.
            sig_i = tanh_g = sig_o = m1 = None
            kcnt = [0, 0, 0, 0]
            for n, k in PAIR_ORDER:
                gp = gps[n]
                c0 = n * 512
                kcnt[n] += 1
                st = (kcnt[n] == 4)
                hk = h2T[:, k % 2, k // 2, :]
                nc.tensor.matmul(gp[0:64, :], hk,
                                 whhT[:, k, c0:c0 + 256],
                                 start=False, stop=st, tile_position=(0, 0))
                nc.tensor.matmul(gp[64:128, :], hk,
                                 whhT[:, k, c0 + 256:c0 + 512],
                                 start=False, stop=st, tile_position=(0, 64))
                if not st:
                    continue
                if n == 0:
                    sig_i = workpool.tile([128, 256], BF16, tag="sig_i")
                    nc.scalar.activation(sig_i, gps[0], SIG)
                elif n == 1:
                    tanh_g = workpool.tile([128, 256], BF16, tag="tanh_g")
                    nc.scalar.activation(tanh_g, gps[1], TANH)
                    m1 = workpool.tile([128, 256], F32, tag="m1")
                    nc.vector.tensor_mul(m1, sig_i, tanh_g)
                elif n == 2:
                    sig_f = workpool.tile([128, 256], BF16, tag="sig_f")
                    nc.scalar.activation(sig_f, gps[2], SIG)
                    m2 = workpool.tile([128, 256], F32, tag="m2")
                    nc.vector.tensor_mul(m2, sig_f, cF)
                else:
                    sig_o = workpool.tile([128, 256], BF16, tag="sig_o")
                    nc.scalar.activation(sig_o, gps[3], SIG)
            # next step's folds go ahead of the tail transposes in the PE
            # FIFO so the PE stays busy while the c-path chain completes
            if t + 1 < T:
                next_gps = emit_folds(t + 1)
            # --- A/B-split tail: each half of the c/h chain unblocks half
            # of the next step's W_hh pairs ---
            cF2 = statepool.tile([128, 256], F32, tag="cF")
            nc.vector.tensor_add(cF2[:, 0:128], m1[:, 0:128], m2[:, 0:128])
            nc.vector.tensor_add(cF2[:, 128:256], m1[:, 128:256],
                                 m2[:, 128:256])
            cF = cF2
            tc2 = workpool.tile([128, 256], BF16, tag="tc2")
            nc.scalar.activation(tc2[:, 0:128], cF[:, 0:128], TANH)
            nc.scalar.activation(tc2[:, 128:256], cF[:, 128:256], TANH)
            # transpose sig(o) and tanh(c2) via REGULAR identity matmuls
            # (counts as PE-busy for HAM, exact, out=f32 psum):
            #   xT[:, s*64:(s+1)*64] = x-chunk (2s + a) in h-major form
            soT = trpsum.tile([128, 2, 128], F32, tag="soT")
            nc.tensor.matmul(soT[:, 0, :], sig_o[:, 0:128], ident,
                             start=True, stop=True)
            nc.tensor.matmul(soT[:, 1, :], sig_o[:, 128:256], ident,
                             start=True, stop=True)
            # junk matmul hooked on sig(o): pads the PE gap while tanh runs
            nc.tensor.matmul(junk2[0:64, :], sig_o[:, 0:64], junkin[:, 0:256],
                             start=True, stop=True)
            soT_sb = workpool.tile([128, 256], BF16, tag="soT_sb")
            nc.vector.tensor_copy(soT_sb[:, 0:128], soT[:, 0, :])
            nc.vector.tensor_copy(soT_sb[:, 128:256], soT[:, 1, :])
            tcTA = trpsum.tile([128, 128], F32, tag="tcTA")
            nc.tensor.matmul(tcTA, tc2[:, 0:128], ident,
                             start=True, stop=True)
            # junk matmul hooked on tanh(c2) A-half: pads until mul-A lands
            nc.tensor.matmul(junk2[0:64, :], tc2[:, 0:64], junkin[:, 0:256],
                             start=True, stop=True)
            tcTB = trpsum.tile([128, 128], F32, tag="tcTB")
            nc.tensor.matmul(tcTB, tc2[:, 128:256], ident,
                             start=True, stop=True)
            h2T = statepool.tile([128, 2, 2, 64], BF16, tag="hT")
            nc.vector.tensor_mul(h2T[:, 0, :, :].rearrange("p s b -> p (s b)"),
                                 soT_sb[:, 0:128], tcTA)
            nc.vector.tensor_mul(h2T[:, 1, :, :].rearrange("p s b -> p (s b)"),
                                 soT_sb[:, 128:256], tcTB)
            # stash this core's 8 rows for phase 3 (b-major cols b*T + t):
            # h-chunks {0,2} come from A (a=0), {1,3} from B (a=1)
            nc.vector.tensor_copy(HT[:, 0:4:2, t:BLOC * T:T],
                                  h2T[:, 0, :, 0:BLOC])
            nc.scalar.copy(HT[:, 1:4:2, t:BLOC * T:T],
                           h2T[:, 1, :, 0:BLOC])
            if t % 2 == 0 and t // 2 + 6 < N_GATHER:
                emit_gather(t // 2 + 6)
            gps = next_gps if t + 1 < T else None
        # keep the junk bank live so the pad matmuls aren't dead code
        jout2 = workpool.tile([1, 8], F32, tag="jout2")
        nc.vector.tensor_copy(jout2, junk2[0:1, 0:8])
        nc.sync.dma_start(out=junk_dram[:], in_=jout2)

    # ---- phase 3: attention + logits for this core's 8 batch rows -----
    NB = BLOC * T  # 1200 columns, ordered b*T + t
    p3pool = tc.alloc_tile_pool(name="p3", bufs=1)
    mappedT = p3pool.tile([128, 4, NB], BF16)

    # stage A: mappedT[h', bt] = sum_h W_hm[h', h] * hT[h, bt]
    with tc.tile_pool(name="bigpA", bufs=1, space="PSUM") as bigpsum, \
         tc.tile_pool(name="p3m", bufs=3) as p3m:
        NCH = [(j * 300, 300) for j in range(4)]
        for m in range(4):
            ps = bigpsum.tile([128, 4, 512], F32, tag="big")
            for k in range(4):
                for j, (n0, nsz) in enumerate(NCH):
                    nc.tensor.matmul(
                        ps[:, j, :nsz], whmT[:, k, m * 128:(m + 1) * 128],
                        HT[:, k, n0:n0 + nsz],
                        start=(k == 0), stop=(k == 3))
            for j, (n0, nsz) in enumerate(NCH):
                _copy(nc, j, mappedT[:, m, n0:n0 + nsz], ps[:, j, :nsz])

    ctxT = p3pool.tile([128, 4, NB], BF16)
    outT = p3pool.tile([128, 4, NB], BF16)

    # stage B: attention + context (staged; apsum/wpsum double-buffered)
    with tc.tile_pool(name="p3w", bufs=4) as p3work, \
         tc.tile_pool(name="bigp", bufs=1, space="PSUM") as bigpsum, \
         tc.tile_pool(name="apsum", bufs=2, space="PSUM") as apsum, \
         tc.tile_pool(name="wpsum", bufs=2, space="PSUM") as wpsum:
        for b in range(BLOC):
            wT = p3work.tile([128, 2, T], BF16, tag="wT")
            for th in range(2):
                t0 = th * 75
                pa = apsum.tile([75, 256], F32, tag="attn")
                for k in range(4):
                    nc.tensor.matmul(
                        pa, mappedT[:, k, b * T + t0:b * T + t0 + 75],
                        cnnT_sb[:, b, k, :], start=(k == 0), stop=(k == 3))
                amax = p3work.tile([75, 1], F32, tag="amax")
                nc.vector.tensor_reduce(amax, pa, X, MAXOP, negate=True)
                w_sb = p3work.tile([75, 256], BF16, tag="w_sb")
                sumex = p3work.tile([75, 1], F32, tag="sumex")
                nc.scalar.activation(w_sb, pa, EXP, bias=amax,
                                     accum_out=sumex)
                rs = p3work.tile([75, 1], F32, tag="rs")
                nc.vector.reciprocal(rs, sumex)
                nc.vector.tensor_scalar_mul(w_sb, w_sb, rs)
                pw = wpsum.tile([128, 2, 76], BF16, tag="wtp")
                for k in range(2):
                    nc.tensor.transpose(
                        pw[:, k, :75], w_sb[:, k * 128:(k + 1) * 128],
                        ident[:75, :75])
                nc.scalar.copy(wT[:, :, t0:t0 + 75], pw[:, :, :75])
            # ctxT[h, t] = sum_l cnn[b, l, h] * w[t, l]
            pc = bigpsum.tile([128, 4, 512], F32, tag="big")
            for k in range(2):
                for m in range(4):
                    nc.tensor.matmul(
                        pc[:, m, :T], cnn_sb[:, b, k, m * 128:(m + 1) * 128],
                        wT[:, k, :], start=(k == 0), stop=(k == 1))
            for m in range(4):
                _copy(nc, m, ctxT[:, m, b * T:(b + 1) * T], pc[:, m, :T])

        # outT[h', bt] = sum_k W_om.T[k, h'] * [ctxT; HT][k, bt]
        for m in range(4):
            ps = bigpsum.tile([128, 4, 512], F32, tag="big")
            for k in range(8):
                src = ctxT if k < 4 else HT
                kk = k % 4
                for j, (n0, nsz) in enumerate(NCH):
                    nc.tensor.matmul(
                        ps[:, j, :nsz], womT[:, k, m * 128:(m + 1) * 128],
                        src[:, kk, n0:n0 + nsz],
                        start=(k == 0), stop=(k == 7))
            for j, (n0, nsz) in enumerate(NCH):
                _copy(nc, j, outT[:, m, n0:n0 + nsz], ps[:, j, :nsz])

    # stage C: logits + log_softmax (lpsum triple-buffered so the next
    # group's matmuls overlap this group's softmax chain)
    with tc.tile_pool(name="p3l", bufs=4) as p3lw, \
         tc.tile_pool(name="lpsum", bufs=3, space="PSUM") as lpsum, \
         tc.tile_pool(name="outp", bufs=3) as outpool:
        for b in range(BLOC):
            for th in range(2):
                t0 = th * 75
                pl = lpsum.tile([75, 1024], F32, tag="logit")
                for k in range(4):
                    for v0, vs in ((0, 512), (512, 488)):
                        nc.tensor.matmul(
                            pl[:, v0:v0 + vs],
                            outT[:, k, b * T + t0:b * T + t0 + 75],
                            wlogT[:, k, v0:v0 + vs],
                            start=(k == 0), stop=False)
                for v0, vs in ((0, 512), (512, 488)):
                    nc.tensor.matmul(
                        pl[:, v0:v0 + vs], ones2[0:1, :75], blog[:, v0:v0 + vs],
                        start=False, stop=True)
                ex = p3lw.tile([75, V], BF16, tag="ex")
                sume = p3lw.tile([75, 1], F32, tag="sume")
                nc.scalar.activation(ex[:, :V], pl[:, :V], EXP,
                                     accum_out=sume)
                lns = p3lw.tile([75, 1], F32, tag="lns")
                nc.scalar.activation(lns, sume, LN)
                outt = outpool.tile([75, V], F32, tag="outt")
                nc.vector.tensor_scalar(
                    out=outt[:, :512], in0=pl[:, :512], scalar1=lns,
                    scalar2=None, op0=SUB)
                nc.vector.tensor_scalar(
                    out=outt[:, 512:V], in0=pl[:, 512:V], scalar1=lns,
                    scalar2=None, op0=SUB)
                nc.sync.dma_start(out=out_d.ap()[b, t0:t0 + 75, :], in_=outt)

    p3pool.release()
    singles.release()


def _prep_inputs(cnn_feats, seq, embed_table, W_ih, b_ih, W_hh, b_hh,
                 W_hm, W_om, W_logit, b_logit):
    bf = NPBF16
    perm = np.r_[0:512, 1024:1536, 512:1024, 1536:2048]  # [i|g|f|o]

    def pack_T(w):  # [out, in] -> [in//128, 128, out] transposed chunks
        wt = np.ascontiguousarray(w.T.astype(bf))
        return wt.reshape(-1, 128, w.shape[0])

    common = {
        "whhT": pack_T(np.asarray(W_hh)[perm]),
        "wihT": pack_T(np.asarray(W_ih)[perm]),
        "embT": pack_T(np.asarray(embed_table)),
        "bias2": np.stack([np.asarray(b_ih)[perm],
                           np.asarray(b_hh)[perm]]).astype(bf),
        "whmT": pack_T(np.asarray(W_hm)),
        "womT": pack_T(np.asarray(W_om)),
        "wlogT": pack_T(np.asarray(W_logit)),
        "blog": np.asarray(b_logit).astype(bf).reshape(1, V),
    }
    cnn = np.asarray(cnn_feats).astype(bf)            # [64, 256, 512]
    cnnK = cnn.reshape(B, 2, 128, H)
    cnnT = np.ascontiguousarray(cnn.transpose(0, 2, 1)).reshape(B, 4, 128, L)
    seq_np = np.asarray(seq).astype(np.int64)         # [64, 150]

    in_maps = []
    for k in range(NCORES):
        order = (np.arange(B) + BLOC * k) % B
        flat = seq_np[order].T.reshape(-1)            # t-major [9600]
        wrapped = np.zeros((128, T * B // 16), np.int16)
        wrapped[:16, :] = flat.reshape(-1, 16).T
        m = dict(common)
        m["cnn"] = np.ascontiguousarray(cnnK[BLOC * k:BLOC * (k + 1)])
        m["cnnT"] = np.ascontiguousarray(cnnT[BLOC * k:BLOC * (k + 1)])
        m["idx"] = wrapped
        in_maps.append(m)
    return in_maps


_CACHE = {}


def kernel(**inputs):
    in_maps = _prep_inputs(**inputs)
    if "nc" not in _CACHE:
        _CACHE["nc"] = build_program()
    nc = _CACHE["nc"]
    trace = bool(int(os.environ.get("KERNEL_TRACE", "0")))
    res = run_bass_kernel_spmd(
        nc, in_maps, core_ids=list(range(NCORES)), trace=trace)
    if trace:
        print(f"HW exec time: {res.exec_time_ns} ns")
    out = np.concatenate([r["out"] for r in res.results], axis=0)  # [64,150,1000]
    return out.astype(np.float32)


if __name__ == "__main__":
    sys.path.insert(0, "/root/problem")
    import reference
    inputs = {k: np.asarray(v) for k, v in reference.setup_inputs().items()}
    got = kernel(**inputs)
    print("out shape", got.shape)


# revision 27
# speedup vs baseline: 1.0065x; 1.0065x over previous
"""AttentionDecoder Trainium2 kernel (v2).

Strategy (8 NeuronCores, zero collectives):
  - The LSTM recurrence (T=150 serial steps) is run REDUNDANTLY with the full
    B=64 batch on every core: PE streaming time is independent of the matmul
    M dim, so full-batch costs the same as 1/8-batch, and we avoid all
    per-step communication.
  - Each core's batch ORDER is rotated (core k leads with rows 8k..8k+8), so
    the phase-3 attention/logits work (which depends only on the stored h2
    sequence) is batch-sharded 8 ways using identical SPMD code.
  - x-projection is folded into a precomputed table P = relu(E) @ W_ih.T + b
    (only 1000 vocab rows), gathered per-step via dma_gather.
  - All matmuls in bf16 (1 cyc/row vs 4 for fp32); cell state c kept in f32.

v2 phase-2 redesign (vs the v1 baseline):
  - FOLDED gate layout: each 512-wide gate is computed as [128, 256] via a
    column-tiled matmul pair (tile_position (.,0) / (.,64)), so ACT/DVE use
    all 128 lanes (2x fewer cycles) and only TWO PE transposes per step are
    needed to rebuild hT (each transpose yields two h-chunks).
  - Per-gate PSUM banks + eager per-gate activations in order [i,g,f,o]:
    the c-path (m1/m2/c2/tanh) overlaps the f/o matmul groups, so the
    serial tail is just sigmoid(o) -> h2 -> transpose -> copy.
  - No filler matmuls; the dense 40-MM burst per step keeps HAM warm.
"""

import os
import sys

import numpy as np

sys.path.insert(0, "/opt/trn_rl_repo")

import ml_dtypes  # noqa: E402

import concourse.bass as bass  # noqa: E402
import concourse.bacc as bacc  # noqa: E402
import concourse.mybir as mybir  # noqa: E402
import concourse.tile as tile  # noqa: E402
from concourse.bass_utils import run_bass_kernel_spmd  # noqa: E402
from concourse.masks import make_identity  # noqa: E402


def _install_ntff_hook():
    """The agent image's antenv lacks axon_hooks; shim it so trace=True can
    reach the ctypes NTFF profiling hook in trn_agent_boot."""
    import types
    if "antenv.axon_hooks" in sys.modules:
        return
    mod = types.ModuleType("antenv.axon_hooks")
    state = {"hook": None}
    mod.set_axon_ntff_profile_hook = lambda h: state.__setitem__("hook", h)
    mod.get_axon_ntff_profile_hook = lambda: state["hook"]
    sys.modules["antenv.axon_hooks"] = mod
    try:
        import antenv
        antenv.axon_hooks = mod
    except ImportError:
        pass
    try:
        from trn_agent_boot.trn_boot import _ntff_profile_via_ctypes
        mod.set_axon_ntff_profile_hook(
            _ntff_profile_via_ctypes("/opt/axon/libaxon_pjrt.so"))
    except Exception:
        pass


_install_ntff_hook()

BF16 = mybir.dt.bfloat16
F32 = mybir.dt.float32
NPBF16 = ml_dtypes.bfloat16

B, T, L, H, E, V = 64, 150, 256, 512, 512, 1000
G = 4 * H  # 2048
NCORES = 8
BLOC = B // NCORES  # 8
SIG = mybir.ActivationFunctionType.Sigmoid
TANH = mybir.ActivationFunctionType.Tanh
EXP = mybir.ActivationFunctionType.Exp
LN = mybir.ActivationFunctionType.Ln
RELU = mybir.ActivationFunctionType.Relu
ADD = mybir.AluOpType.add
SUB = mybir.AluOpType.subtract
MAXOP = mybir.AluOpType.max
X = mybir.AxisListType.X

N_GATHER = T * B // 128  # 75 gathers of 128 rows (2 steps each)


def build_program():
    nc = bacc.Bacc(
        "TRN2",
        target_bir_lowering=False,
        debug=False,
        num_swdge_queues=4,
    )

    # ---- I/O -----------------------------------------------------------
    whhT_d = nc.dram_tensor("whhT", [4, 128, G], BF16, kind="ExternalInput")
    wihT_d = nc.dram_tensor("wihT", [4, 128, G], BF16, kind="ExternalInput")
    embT_d = nc.dram_tensor("embT", [4, 128, V], BF16, kind="ExternalInput")
    bias2_d = nc.dram_tensor("bias2", [2, G], BF16, kind="ExternalInput")
    whmT_d = nc.dram_tensor("whmT", [4, 128, H], BF16, kind="ExternalInput")
    womT_d = nc.dram_tensor("womT", [8, 128, H], BF16, kind="ExternalInput")
    wlogT_d = nc.dram_tensor("wlogT", [4, 128, V], BF16, kind="ExternalInput")
    blog_d = nc.dram_tensor("blog", [1, V], BF16, kind="ExternalInput")
    cnn_d = nc.dram_tensor("cnn", [BLOC, 2, 128, H], BF16, kind="ExternalInput")
    cnnT_d = nc.dram_tensor("cnnT", [BLOC, 4, 128, L], BF16, kind="ExternalInput")
    idx_d = nc.dram_tensor("idx", [128, T * B // 16], mybir.dt.int16,
                           kind="ExternalInput")
    out_d = nc.dram_tensor("out", [BLOC, T, V], F32, kind="ExternalOutput")

    with tile.TileContext(nc) as tc:
        _body(nc, tc, whhT_d, wihT_d, embT_d, bias2_d, whmT_d, womT_d,
              wlogT_d, blog_d, cnn_d, cnnT_d, idx_d, out_d)
    nc.compile()
    return nc


def _copy(nc, j, out, in_):
    # psum -> sbuf copies: only DVE/ACT may read PSUM
    if j % 2 == 1:
        nc.scalar.copy(out, in_)
    else:
        nc.vector.tensor_copy(out, in_)


def _body(nc, tc, whhT_d, wihT_d, embT_d, bias2_d, whmT_d, womT_d,
          wlogT_d, blog_d, cnn_d, cnnT_d, idx_d, out_d):
    singles = tc.alloc_tile_pool(name="singles", bufs=1)

    # ---- resident constants -------------------------------------------
    whhT = singles.tile([128, 4, G], BF16)
    idx_sb = singles.tile([128, T * B // 16], mybir.dt.int16)
    nc.sync.dma_start(out=idx_sb, in_=idx_d.ap())
    ident = singles.tile([128, 128], BF16)
    make_identity(nc, ident)
    ones2 = singles.tile([2, 128], BF16)
    nc.vector.memset(ones2, 1.0)
    bias2 = singles.tile([2, G], BF16)
    nc.sync.dma_start(out=bias2, in_=bias2_d.ap())
    junkin = singles.tile([128, 512], BF16)
    nc.vector.memset(junkin, 0.0)

    # phase-3 resident weights / data (tiles allocated here; their DMAs are
    # deferred until after the gather kickoff so they don't gate phase 0)
    whmT = singles.tile([128, 4, H], BF16)
    womT = singles.tile([128, 8, H], BF16)
    wlogT = singles.tile([128, 4, V], BF16)
    blog = singles.tile([1, V], BF16)
    cnn_sb = singles.tile([128, BLOC, 2, H], BF16)   # [l%128, b, l//128, h]
    cnnT_sb = singles.tile([128, BLOC, 4, L], BF16)  # [h%128, b, h//128, l]

    # HT_sb collects transposed h2 for this core's 8 batch rows:
    # [h%128, h//128, b*T + t]
    HT = singles.tile([128, 4, BLOC * T], BF16)

    # ---- phase 0: P = relu(embT).T @ W_ih.T + (b_ih + b_hh) ------------
    drampool = tc.alloc_tile_pool(name="dram", bufs=1, space="DRAM")
    P_dram = drampool.tile([V, G], BF16)
    with tc.tile_pool(name="p0", bufs=1) as p0pool, \
         tc.tile_pool(name="p0psum", bufs=2, space="PSUM") as p0psum, \
         tc.tile_pool(name="p0junk", bufs=1, space="PSUM") as p0junk, \
         tc.tile_pool(name="p0out", bufs=3) as p0out:
        wihT = p0pool.tile([128, 4, G], BF16)
        embT = p0pool.tile([128, 4, V], BF16)
        for k in range(4):
            nc.sync.dma_start(out=wihT[:, k, :], in_=wihT_d.ap()[k])
            nc.sync.dma_start(out=embT[:, k, :], in_=embT_d.ap()[k])
            nc.scalar.activation(embT[:, k, :], embT[:, k, :], RELU)
        # whhT is not needed until the recurrence starts; keep it off the
        # phase-0 critical DMA path
        for k in range(4):
            nc.sync.dma_start(out=whhT[:, k, :], in_=whhT_d.ap()[k])
        # warm-up matmuls: keep the PE (and its HAM clock gate) busy while
        # the embT/wihT DMAs land, so phase 0 runs at 2.4 GHz
        junkp = p0junk.tile([128, 512], F32)
        for _ in range(36):
            nc.tensor.matmul(junkp[0:64, :], ident[:, 0:64], junkin,
                             start=True, stop=True)
        jout = p0out.tile([1, 8], F32, tag="jout")
        nc.vector.tensor_copy(jout, junkp[0:1, 0:8])
        junk_dram = drampool.tile([1, 8], F32)
        nc.sync.dma_start(out=junk_dram[:], in_=jout)
        for m in range(8):  # vocab chunks: 7*128 + 104
            mr = min(128, V - m * 128)
            pt = p0out.tile([128, G], BF16)
            for half in range(2):  # 2-bank psum groups so bufs=2 fits
                ps = p0psum.tile([128, 2, 512], F32)
                for k in range(4):
                    for j in range(2):
                        n = half * 2 + j
                        nc.tensor.matmul(
                            ps[:mr, j, :],
                            embT[:, k, m * 128:m * 128 + mr],
                            wihT[:, k, n * 512:(n + 1) * 512],
                            start=(k == 0), stop=False)
                for j in range(2):
                    n = half * 2 + j
                    nc.tensor.matmul(
                        ps[:mr, j, :], ones2[:, :mr],
                        bias2[:, n * 512:(n + 1) * 512],
                        start=False, stop=True)
                for j in range(2):
                    n = half * 2 + j
                    _copy(nc, n, pt[:mr, n * 512:(n + 1) * 512], ps[:mr, j, :])
            nc.sync.dma_start(out=P_dram[m * 128:m * 128 + mr, :], in_=pt[:mr, :])

    # ---- phase 2: the recurrence (folded [128, 256] gate layout) -------
    # Logical [64, 512] per-gate tensors are stored folded as [128, 256]:
    # FX[p, c] = X[p % 64, (p // 64) * 256 + c].  All elementwise ops are
    # layout-agnostic; the PE transpose of a [128, 128] slice of folded h2
    # yields TWO h-major hT chunks at once.
    with tc.tile_pool(name="gx", bufs=6) as gxpool, \
         tc.tile_pool(name="state", bufs=2) as statepool, \
         tc.tile_pool(name="work", bufs=3) as workpool, \
         tc.tile_pool(name="gpsum", bufs=1, space="PSUM") as gpsum, \
         tc.tile_pool(name="trpsum", bufs=1, space="PSUM") as trpsum:

        gx_tiles = [None] * N_GATHER
        n128 = nc.gpsimd.to_reg(128)

        def emit_gather(g):
            gxt = gxpool.tile([128, 2048], BF16, tag="gx")
            nc.gpsimd.dma_gather(
                out_ap=gxt.rearrange("p (o f) -> p o f", o=1),
                in_ap=P_dram[:],
                idxs_ap=idx_sb[:, g * 8:(g + 1) * 8],
                num_idxs=128,
                num_idxs_reg=n128,
                elem_size=G,
                queue_num=g % 4,
            )
            gx_tiles[g] = gxt

        for g in range(N_GATHER):
            emit_gather(g)

        # deferred phase-3 constant DMAs: they arrive during the recurrence
        for k in range(4):
            nc.sync.dma_start(out=whmT[:, k, :], in_=whmT_d.ap()[k])
            nc.sync.dma_start(out=wlogT[:, k, :], in_=wlogT_d.ap()[k])
        for k in range(8):
            nc.sync.dma_start(out=womT[:, k, :], in_=womT_d.ap()[k])
        nc.sync.dma_start(out=blog, in_=blog_d.ap())
        for b in range(BLOC):
            for k in range(2):
                nc.sync.dma_start(out=cnn_sb[:, b, k, :], in_=cnn_d.ap()[b, k])
            for k in range(4):
                nc.sync.dma_start(out=cnnT_sb[:, b, k, :], in_=cnnT_d.ap()[b, k])

        # h state lives TRANSPOSED (h-major) as [128, 2(A/B), 2(sub), 64]:
        # h-chunk k is h2T[:, k % 2, k // 2, :].  c stays batch-folded f32.
        h2T = statepool.tile([128, 2, 2, 64], BF16, tag="hT")
        nc.vector.memset(h2T, 0.0)
        cF = statepool.tile([128, 256], F32, tag="cF")
        nc.vector.memset(cF, 0.0)

        def emit_folds(t):
            """Fold the gathered x-projection for step t into fresh gate
            banks (start=True).  No hT dependency, so these run during the
            previous step's tail."""
            po = (t % 2) * 64
            gx = gx_tiles[t // 2]
            idg = ident[po:po + 64, po:po + 64]
            gps = []
            for n in range(4):
                gp = gpsum.tile([128, 256], F32, tag=f"g{n}")
                c0 = n * 512
                nc.tensor.matmul(gp[0:64, :], idg,
                                 gx[po:po + 64, c0:c0 + 256],
                                 start=True, stop=False, tile_position=(po, 0))
                nc.tensor.matmul(gp[64:128, :], idg,
                                 gx[po:po + 64, c0 + 256:c0 + 512],
                                 start=True, stop=False, tile_position=(po, 64))
                gps.append(gp)
            return gps

        # Per-gate W_hh pair order: gates complete early->late as
        # [i, g, f, o] (for the eager ACT chain), while k in {0,2} pairs
        # lead so the A-half of the previous step's h2T unblocks the burst
        # ~0.4us before the B-half is ready.
        PAIR_ORDER = [(0, 0), (0, 2), (1, 0), (1, 2), (0, 1), (0, 3),
                      (1, 1), (1, 3), (2, 0), (2, 2), (3, 0), (3, 2),
                      (2, 1), (2, 3), (3, 1), (3, 3)]

        junk2 = trpsum.tile([128, 256], F32, tag="junk2")
        gps = emit_folds(0)
        for t in range(T):
            # Gate order along G: [i | g | f | o] (see perm in _prep_inputs).
            # Eager per-gate ACT lets the c-path overlap the f/o MM groups.
            sig_i = tanh_g = sig_o = m1 = None
            kcnt = [0, 0, 0, 0]
            for n, k in PAIR_ORDER:
                gp = gps[n]
                c0 = n * 512
                kcnt[n] += 1
                st = (kcnt[n] == 4)
                hk = h2T[:, k % 2, k // 2, :]
                nc.tensor.matmul(gp[0:64, :], hk,
                                 whhT[:, k, c0:c0 + 256],
                                 start=False, stop=st, tile_position=(0, 0))
                nc.tensor.matmul(gp[64:128, :], hk,
                                 whhT[:, k, c0 + 256:c0 + 512],
                                 start=False, stop=st, tile_position=(0, 64))
                if not st:
                    continue
                if n == 0:
                    sig_i = workpool.tile([128, 256], BF16, tag="sig_i")
                    nc.scalar.activation(sig_i, gps[0], SIG)
                elif n == 1:
                    tanh_g = workpool.tile([128, 256], BF16, tag="tanh_g")
                    nc.scalar.activation(tanh_g, gps[1], TANH)
                    m1 = workpool.tile([128, 256], F32, tag="m1")
                    nc.vector.tensor_mul(m1, sig_i, tanh_g)
                elif n == 2:
                    sig_f = workpool.tile([128, 256], BF16, tag="sig_f")
                    nc.scalar.activation(sig_f, gps[2], SIG)
                    m2 = workpool.tile([128, 256], F32, tag="m2")
                    nc.vector.tensor_mul(m2, sig_f, cF)
                else:
                    sig_o = workpool.tile([128, 256], BF16, tag="sig_o")
                    nc.scalar.activation(sig_o, gps[3], SIG)
            # next step's folds go ahead of the tail transposes in the PE
            # FIFO so the PE stays busy while the c-path chain completes
            if t + 1 < T:
                next_gps = emit_folds(t + 1)
            # --- A/B-split tail: each half of the c/h chain unblocks half
            # of the next step's W_hh pairs ---
            cF2 = statepool.tile([128, 256], F32, tag="cF")
            nc.vector.tensor_add(cF2[:, 0:128], m1[:, 0:128], m2[:, 0:128])
            nc.vector.tensor_add(cF2[:, 128:256], m1[:, 128:256],
                                 m2[:, 128:256])
            cF = cF2
            tc2 = workpool.tile([128, 256], BF16, tag="tc2")
            nc.scalar.activation(tc2[:, 0:128], cF[:, 0:128], TANH)
            nc.scalar.activation(tc2[:, 128:256], cF[:, 128:256], TANH)
            # transpose sig(o) and tanh(c2) via REGULAR identity matmuls
            # (counts as PE-busy for HAM, exact, out=f32 psum):
            #   xT[:, s*64:(s+1)*64] = x-chunk (2s + a) in h-major form
            soT = trpsum.tile([128, 2, 128], F32, tag="soT")
            nc.tensor.matmul(soT[:, 0, :], sig_o[:, 0:128], ident,
                             start=True, stop=True)
            nc.tensor.matmul(soT[:, 1, :], sig_o[:, 128:256], ident,
                             start=True, stop=True)
            # junk matmul hooked on sig(o): pads the PE gap while tanh runs
            nc.tensor.matmul(junk2[0:64, :], sig_o[:, 0:64], junkin[:, 0:256],
                             start=True, stop=True)
            soT_sb = workpool.tile([128, 256], BF16, tag="soT_sb")
            nc.vector.tensor_copy(soT_sb[:, 0:128], soT[:, 0, :])
            nc.vector.tensor_copy(soT_sb[:, 128:256], soT[:, 1, :])
            tcTA = trpsum.tile([128, 128], F32, tag="tcTA")
            nc.tensor.matmul(tcTA, tc2[:, 0:128], ident,
                             start=True, stop=True)
            # junk matmul hooked on tanh(c2) A-half: pads until mul-A lands
            nc.tensor.matmul(junk2[0:64, :], tc2[:, 0:64], junkin[:, 0:256],
                             start=True, stop=True)
            tcTB = trpsum.tile([128, 128], F32, tag="tcTB")
            nc.tensor.matmul(tcTB, tc2[:, 128:256], ident,
                             start=True, stop=True)
            h2T = statepool.tile([128, 2, 2, 64], BF16, tag="hT")
            nc.vector.tensor_mul(h2T[:, 0, :, :].rearrange("p s b -> p (s b)"),
                                 soT_sb[:, 0:128], tcTA)
            nc.vector.tensor_mul(h2T[:, 1, :, :].rearrange("p s b -> p (s b)"),
                                 soT_sb[:, 128:256], tcTB)
            # stash this core's 8 rows for phase 3 (b-major cols b*T + t):
            # h-chunks {0,2} come from A (a=0), {1,3} from B (a=1)
            nc.vector.tensor_copy(HT[:, 0:4:2, t:BLOC * T:T],
                                  h2T[:, 0, :, 0:BLOC])
            nc.vector.tensor_copy(HT[:, 1:4:2, t:BLOC * T:T],
                                  h2T[:, 1, :, 0:BLOC])
            gps = next_gps if t + 1 < T else None
        # keep the junk bank live so the pad matmuls aren't dead code
        jout2 = workpool.tile([1, 8], F32, tag="jout2")
        nc.vector.tensor_copy(jout2, junk2[0:1, 0:8])
        nc.sync.dma_start(out=junk_dram[:], in_=jout2)

    # ---- phase 3: attention + logits for this core's 8 batch rows -----
    NB = BLOC * T  # 1200 columns, ordered b*T + t
    p3pool = tc.alloc_tile_pool(name="p3", bufs=1)
    mappedT = p3pool.tile([128, 4, NB], BF16)

    # stage A: mappedT[h', bt] = sum_h W_hm[h', h] * hT[h, bt]
    with tc.tile_pool(name="bigpA", bufs=1, space="PSUM") as bigpsum, \
         tc.tile_pool(name="p3m", bufs=3) as p3m:
        NCH = [(j * 300, 300) for j in range(4)]
        for m in range(4):
            ps = bigpsum.tile([128, 4, 512], F32, tag="big")
            for k in range(4):
                for j, (n0, nsz) in enumerate(NCH):
                    nc.tensor.matmul(
                        ps[:, j, :nsz], whmT[:, k, m * 128:(m + 1) * 128],
                        HT[:, k, n0:n0 + nsz],
                        start=(k == 0), stop=(k == 3))
            for j, (n0, nsz) in enumerate(NCH):
                _copy(nc, j, mappedT[:, m, n0:n0 + nsz], ps[:, j, :nsz])

    ctxT = p3pool.tile([128, 4, NB], BF16)
    outT = p3pool.tile([128, 4, NB], BF16)

    # stage B: attention + context (staged; apsum/wpsum double-buffered)
    with tc.tile_pool(name="p3w", bufs=4) as p3work, \
         tc.tile_pool(name="bigp", bufs=1, space="PSUM") as bigpsum, \
         tc.tile_pool(name="apsum", bufs=2, space="PSUM") as apsum, \
         tc.tile_pool(name="wpsum", bufs=2, space="PSUM") as wpsum:
        for b in range(BLOC):
            wT = p3work.tile([128, 2, T], BF16, tag="wT")
            for th in range(2):
                t0 = th * 75
                pa = apsum.tile([75, 256], F32, tag="attn")
                for k in range(4):
                    nc.tensor.matmul(
                        pa, mappedT[:, k, b * T + t0:b * T + t0 + 75],
                        cnnT_sb[:, b, k, :], start=(k == 0), stop=(k == 3))
                amax = p3work.tile([75, 1], F32, tag="amax")
                nc.vector.tensor_reduce(amax, pa, X, MAXOP, negate=True)
                w_sb = p3work.tile([75, 256], BF16, tag="w_sb")
                sumex = p3work.tile([75, 1], F32, tag="sumex")
                nc.scalar.activation(w_sb, pa, EXP, bias=amax,
                                     accum_out=sumex)
                rs = p3work.tile([75, 1], F32, tag="rs")
                nc.vector.reciprocal(rs, sumex)
                nc.vector.tensor_scalar_mul(w_sb, w_sb, rs)
                pw = wpsum.tile([128, 2, 76], BF16, tag="wtp")
                for k in range(2):
                    nc.tensor.transpose(
                        pw[:, k, :75], w_sb[:, k * 128:(k + 1) * 128],
                        ident[:75, :75])
                nc.scalar.copy(wT[:, :, t0:t0 + 75], pw[:, :, :75])
            # ctxT[h, t] = sum_l cnn[b, l, h] * w[t, l]
            pc = bigpsum.tile([128, 4, 512], F32, tag="big")
            for k in range(2):
                for m in range(4):
                    nc.tensor.matmul(
                        pc[:, m, :T], cnn_sb[:, b, k, m * 128:(m + 1) * 128],
                        wT[:, k, :], start=(k == 0), stop=(k == 1))
            for m in range(4):
                _copy(nc, m, ctxT[:, m, b * T:(b + 1) * T], pc[:, m, :T])

        # outT[h', bt] = sum_k W_om.T[k, h'] * [ctxT; HT][k, bt]
        for m in range(4):
            ps = bigpsum.tile([128, 4, 512], F32, tag="big")
            for k in range(8):
                src = ctxT if k < 4 else HT
                kk = k % 4
                for j, (n0, nsz) in enumerate(NCH):
                    nc.tensor.matmul(
                        ps[:, j, :nsz], womT[:, k, m * 128:(m + 1) * 128],
                        src[:, kk, n0:n0 + nsz],
                        start=(k == 0), stop=(k == 7))
            for j, (n0, nsz) in enumerate(NCH):
                _copy(nc, j, outT[:, m, n0:n0 + nsz], ps[:, j, :nsz])

    # stage C: logits + log_softmax (lpsum triple-buffered so the next
    # group's matmuls overlap this group's softmax chain)
    with tc.tile_pool(name="p3l", bufs=4) as p3lw, \
         tc.tile_pool(name="lpsum", bufs=3, space="PSUM") as lpsum, \
         tc.tile_pool(name="outp", bufs=3) as outpool:
        for b in range(BLOC):
            for th in range(2):
                t0 = th * 75
                pl = lpsum.tile([75, 1024], F32, tag="logit")
                for k in range(4):
                    for v0, vs in ((0, 512), (512, 488)):
                        nc.tensor.matmul(
                            pl[:, v0:v0 + vs],
                            outT[:, k, b * T + t0:b * T + t0 + 75],
                            wlogT[:, k, v0:v0 + vs],
                            start=(k == 0), stop=False)
                for v0, vs in ((0, 512), (512, 488)):
                    nc.tensor.matmul(
                        pl[:, v0:v0 + vs], ones2[0:1, :75], blog[:, v0:v0 + vs],
                        start=False, stop=True)
                ex = p3lw.tile([75, V], BF16, tag="ex")
                sume = p3lw.tile([75, 1], F32, tag="sume")
                nc.scalar.activation(ex[:, :V], pl[:, :V], EXP,
                                     accum_out=sume)
                lns = p3lw.tile([75, 1], F32, tag="lns")
                nc.scalar.activation(lns, sume, LN)
                outt = outpool.tile([75, V], F32, tag="outt")
                nc.vector.tensor_scalar(
                    out=outt[:, :512], in0=pl[:, :512], scalar1=lns,
                    scalar2=None, op0=SUB)
                nc.vector.tensor_scalar(
                    out=outt[:, 512:V], in0=pl[:, 512:V], scalar1=lns,
                    scalar2=None, op0=SUB)
                nc.sync.dma_start(out=out_d.ap()[b, t0:t0 + 75, :], in_=outt)

    p3pool.release()
    singles.release()


def _prep_inputs(cnn_feats, seq, embed_table, W_ih, b_ih, W_hh, b_hh,
                 W_hm, W_om, W_logit, b_logit):
    bf = NPBF16
    perm = np.r_[0:512, 1024:1536, 512:1024, 1536:2048]  # [i|g|f|o]

    def pack_T(w):  # [out, in] -> [in//128, 128, out] transposed chunks
        wt = np.ascontiguousarray(w.T.astype(bf))
        return wt.reshape(-1, 128, w.shape[0])

    common = {
        "whhT": pack_T(np.asarray(W_hh)[perm]),
        "wihT": pack_T(np.asarray(W_ih)[perm]),
        "embT": pack_T(np.asarray(embed_table)),
        "bias2": np.stack([np.asarray(b_ih)[perm],
                           np.asarray(b_hh)[perm]]).astype(bf),
        "whmT": pack_T(np.asarray(W_hm)),
        "womT": pack_T(np.asarray(W_om)),
        "wlogT": pack_T(np.asarray(W_logit)),
        "blog": np.asarray(b_logit).astype(bf).reshape(1, V),
    }
    cnn = np.asarray(cnn_feats).astype(bf)            # [64, 256, 512]
    cnnK = cnn.reshape(B, 2, 128, H)
    cnnT = np.ascontiguousarray(cnn.transpose(0, 2, 1)).reshape(B, 4, 128, L)
    seq_np = np.asarray(seq).astype(np.int64)         # [64, 150]

    in_maps = []
    for k in range(NCORES):
        order = (np.arange(B) + BLOC * k) % B
        flat = seq_np[order].T.reshape(-1)            # t-major [9600]
        wrapped = np.zeros((128, T * B // 16), np.int16)
        wrapped[:16, :] = flat.reshape(-1, 16).T
        m = dict(common)
        m["cnn"] = np.ascontiguousarray(cnnK[BLOC * k:BLOC * (k + 1)])
        m["cnnT"] = np.ascontiguousarray(cnnT[BLOC * k:BLOC * (k + 1)])
        m["idx"] = wrapped
        in_maps.append(m)
    return in_maps


_CACHE = {}


def kernel(**inputs):
    in_maps = _prep_inputs(**inputs)
    if "nc" not in _CACHE:
        _CACHE["nc"] = build_program()
    nc = _CACHE["nc"]
    trace = bool(int(os.environ.get("KERNEL_TRACE", "0")))
    res = run_bass_kernel_spmd(
        nc, in_maps, core_ids=list(range(NCORES)), trace=trace)
    if trace:
        print(f"HW exec time: {res.exec_time_ns} ns")
    out = np.concatenate([r["out"] for r in res.results], axis=0)  # [64,150,1000]
    return out.astype(np.float32)


if __name__ == "__main__":
    sys.path.insert(0, "/root/problem")
    import reference
    inputs = {k: np.asarray(v) for k, v in reference.setup_inputs().items()}
    got = kernel(**inputs)
    print("out shape", got.shape)


# revision 28
# speedup vs baseline: 1.2023x; 1.1946x over previous
"""AttentionDecoder Trainium2 kernel (v2).

Strategy (8 NeuronCores, zero collectives):
  - The LSTM recurrence (T=150 serial steps) is run REDUNDANTLY with the full
    B=64 batch on every core: PE streaming time is independent of the matmul
    M dim, so full-batch costs the same as 1/8-batch, and we avoid all
    per-step communication.
  - Each core's batch ORDER is rotated (core k leads with rows 8k..8k+8), so
    the phase-3 attention/logits work (which depends only on the stored h2
    sequence) is batch-sharded 8 ways using identical SPMD code.
  - x-projection is folded into a precomputed table P = relu(E) @ W_ih.T + b
    (only 1000 vocab rows), gathered per-step via dma_gather.
  - All matmuls in bf16 (1 cyc/row vs 4 for fp32); cell state c kept in f32.

v2 phase-2 redesign (vs the v1 baseline):
  - FOLDED gate layout: each 512-wide gate is computed as [128, 256] via a
    column-tiled matmul pair (tile_position (.,0) / (.,64)), so ACT/DVE use
    all 128 lanes (2x fewer cycles) and only TWO PE transposes per step are
    needed to rebuild hT (each transpose yields two h-chunks).
  - Per-gate PSUM banks + eager per-gate activations in order [i,g,f,o]:
    the c-path (m1/m2/c2/tanh) overlaps the f/o matmul groups, so the
    serial tail is just sigmoid(o) -> h2 -> transpose -> copy.
  - No filler matmuls; the dense 40-MM burst per step keeps HAM warm.
"""

import os
import sys

import numpy as np

sys.path.insert(0, "/opt/trn_rl_repo")

import ml_dtypes  # noqa: E402

import concourse.bass as bass  # noqa: E402
import concourse.bacc as bacc  # noqa: E402
import concourse.mybir as mybir  # noqa: E402
import concourse.tile as tile  # noqa: E402
from concourse.bass_utils import run_bass_kernel_spmd  # noqa: E402
from concourse.masks import make_identity  # noqa: E402


def _install_ntff_hook():
    """The agent image's antenv lacks axon_hooks; shim it so trace=True can
    reach the ctypes NTFF profiling hook in trn_agent_boot."""
    import types
    if "antenv.axon_hooks" in sys.modules:
        return
    mod = types.ModuleType("antenv.axon_hooks")
    state = {"hook": None}
    mod.set_axon_ntff_profile_hook = lambda h: state.__setitem__("hook", h)
    mod.get_axon_ntff_profile_hook = lambda: state["hook"]
    sys.modules["antenv.axon_hooks"] = mod
    try:
        import antenv
        antenv.axon_hooks = mod
    except ImportError:
        pass
    try:
        from trn_agent_boot.trn_boot import _ntff_profile_via_ctypes
        mod.set_axon_ntff_profile_hook(
            _ntff_profile_via_ctypes("/opt/axon/libaxon_pjrt.so"))
    except Exception:
        pass


_install_ntff_hook()

BF16 = mybir.dt.bfloat16
F32 = mybir.dt.float32
NPBF16 = ml_dtypes.bfloat16

B, T, L, H, E, V = 64, 150, 256, 512, 512, 1000
G = 4 * H  # 2048
NCORES = 8
BLOC = B // NCORES  # 8
SIG = mybir.ActivationFunctionType.Sigmoid
TANH = mybir.ActivationFunctionType.Tanh
EXP = mybir.ActivationFunctionType.Exp
LN = mybir.ActivationFunctionType.Ln
RELU = mybir.ActivationFunctionType.Relu
ADD = mybir.AluOpType.add
SUB = mybir.AluOpType.subtract
MAXOP = mybir.AluOpType.max
X = mybir.AxisListType.X

N_GATHER = T * B // 128  # 75 gathers of 128 rows (2 steps each)


def build_program():
    nc = bacc.Bacc(
        "TRN2",
        target_bir_lowering=False,
        debug=False,
        num_swdge_queues=4,
    )

    # ---- I/O -----------------------------------------------------------
    whhT_d = nc.dram_tensor("whhT", [4, 128, G], BF16, kind="ExternalInput")
    wihT_d = nc.dram_tensor("wihT", [4, 128, G], BF16, kind="ExternalInput")
    embT_d = nc.dram_tensor("embT", [4, 128, V], BF16, kind="ExternalInput")
    bias2_d = nc.dram_tensor("bias2", [2, G], BF16, kind="ExternalInput")
    whmT_d = nc.dram_tensor("whmT", [4, 128, H], BF16, kind="ExternalInput")
    womT_d = nc.dram_tensor("womT", [8, 128, H], BF16, kind="ExternalInput")
    wlogT_d = nc.dram_tensor("wlogT", [4, 128, V], BF16, kind="ExternalInput")
    blog_d = nc.dram_tensor("blog", [1, V], BF16, kind="ExternalInput")
    cnn_d = nc.dram_tensor("cnn", [BLOC, 2, 128, H], BF16, kind="ExternalInput")
    cnnT_d = nc.dram_tensor("cnnT", [BLOC, 4, 128, L], BF16, kind="ExternalInput")
    idx_d = nc.dram_tensor("idx", [128, T * B // 16], mybir.dt.int16,
                           kind="ExternalInput")
    out_d = nc.dram_tensor("out", [BLOC, T, V], F32, kind="ExternalOutput")

    with tile.TileContext(nc) as tc:
        _body(nc, tc, whhT_d, wihT_d, embT_d, bias2_d, whmT_d, womT_d,
              wlogT_d, blog_d, cnn_d, cnnT_d, idx_d, out_d)
    nc.compile()
    return nc


def _copy(nc, j, out, in_):
    # psum -> sbuf copies: only DVE/ACT may read PSUM
    if j % 2 == 1:
        nc.scalar.copy(out, in_)
    else:
        nc.vector.tensor_copy(out, in_)


def _body(nc, tc, whhT_d, wihT_d, embT_d, bias2_d, whmT_d, womT_d,
          wlogT_d, blog_d, cnn_d, cnnT_d, idx_d, out_d):
    singles = tc.alloc_tile_pool(name="singles", bufs=1)

    # ---- resident constants -------------------------------------------
    whhT = singles.tile([128, 4, G], BF16)
    idx_sb = singles.tile([128, T * B // 16], mybir.dt.int16)
    nc.sync.dma_start(out=idx_sb, in_=idx_d.ap())
    ident = singles.tile([128, 128], BF16)
    make_identity(nc, ident)
    ones2 = singles.tile([2, 128], BF16)
    nc.vector.memset(ones2, 1.0)
    bias2 = singles.tile([2, G], BF16)
    nc.sync.dma_start(out=bias2, in_=bias2_d.ap())
    junkin = singles.tile([128, 512], BF16)
    nc.vector.memset(junkin, 0.0)

    # phase-3 resident weights / data (tiles allocated here; their DMAs are
    # deferred until after the gather kickoff so they don't gate phase 0)
    whmT = singles.tile([128, 4, H], BF16)
    womT = singles.tile([128, 8, H], BF16)
    wlogT = singles.tile([128, 4, V], BF16)
    blog = singles.tile([1, V], BF16)
    cnn_sb = singles.tile([128, BLOC, 2, H], BF16)   # [l%128, b, l//128, h]
    cnnT_sb = singles.tile([128, BLOC, 4, L], BF16)  # [h%128, b, h//128, l]

    # HT_sb collects transposed h2 for this core's 8 batch rows:
    # [h%128, h//128, b*T + t]
    HT = singles.tile([128, 4, BLOC * T], BF16)

    # ---- phase 0: P = relu(embT).T @ W_ih.T + (b_ih + b_hh) ------------
    drampool = tc.alloc_tile_pool(name="dram", bufs=1, space="DRAM")
    P_dram = drampool.tile([V, G], BF16)
    with tc.tile_pool(name="p0", bufs=1) as p0pool, \
         tc.tile_pool(name="p0psum", bufs=2, space="PSUM") as p0psum, \
         tc.tile_pool(name="p0junk", bufs=1, space="PSUM") as p0junk, \
         tc.tile_pool(name="p0out", bufs=3) as p0out:
        wihT = p0pool.tile([128, 4, G], BF16)
        embT = p0pool.tile([128, 4, V], BF16)
        for k in range(4):
            nc.sync.dma_start(out=wihT[:, k, :], in_=wihT_d.ap()[k])
            nc.sync.dma_start(out=embT[:, k, :], in_=embT_d.ap()[k])
            nc.scalar.activation(embT[:, k, :], embT[:, k, :], RELU)
        # whhT is not needed until the recurrence starts; keep it off the
        # phase-0 critical DMA path
        for k in range(4):
            nc.sync.dma_start(out=whhT[:, k, :], in_=whhT_d.ap()[k])
        # warm-up matmuls: keep the PE (and its HAM clock gate) busy while
        # the embT/wihT DMAs land, so phase 0 runs at 2.4 GHz
        junkp = p0junk.tile([128, 512], F32)
        for _ in range(36):
            nc.tensor.matmul(junkp[0:64, :], ident[:, 0:64], junkin,
                             start=True, stop=True)
        jout = p0out.tile([1, 8], F32, tag="jout")
        nc.vector.tensor_copy(jout, junkp[0:1, 0:8])
        junk_dram = drampool.tile([1, 8], F32)
        nc.sync.dma_start(out=junk_dram[:], in_=jout)
        for m in range(8):  # vocab chunks: 7*128 + 104
            mr = min(128, V - m * 128)
            pt = p0out.tile([128, G], BF16)
            for half in range(2):  # 2-bank psum groups so bufs=2 fits
                ps = p0psum.tile([128, 2, 512], F32)
                for k in range(4):
                    for j in range(2):
                        n = half * 2 + j
                        nc.tensor.matmul(
                            ps[:mr, j, :],
                            embT[:, k, m * 128:m * 128 + mr],
                            wihT[:, k, n * 512:(n + 1) * 512],
                            start=(k == 0), stop=False)
                for j in range(2):
                    n = half * 2 + j
                    nc.tensor.matmul(
                        ps[:mr, j, :], ones2[:, :mr],
                        bias2[:, n * 512:(n + 1) * 512],
                        start=False, stop=True)
                for j in range(2):
                    n = half * 2 + j
                    _copy(nc, n, pt[:mr, n * 512:(n + 1) * 512], ps[:mr, j, :])
            nc.sync.dma_start(out=P_dram[m * 128:m * 128 + mr, :], in_=pt[:mr, :])

    # ---- phase 2: the recurrence (folded [128, 256] gate layout) -------
    # Logical [64, 512] per-gate tensors are stored folded as [128, 256]:
    # FX[p, c] = X[p % 64, (p // 64) * 256 + c].  All elementwise ops are
    # layout-agnostic; the PE transpose of a [128, 128] slice of folded h2
    # yields TWO h-major hT chunks at once.
    with tc.tile_pool(name="gx", bufs=6) as gxpool, \
         tc.tile_pool(name="state", bufs=2) as statepool, \
         tc.tile_pool(name="work", bufs=3) as workpool, \
         tc.tile_pool(name="gpsum", bufs=1, space="PSUM") as gpsum, \
         tc.tile_pool(name="trpsum", bufs=1, space="PSUM") as trpsum:

        gx_tiles = [None] * N_GATHER
        n128 = nc.gpsimd.to_reg(128)

        def emit_gather(g):
            gxt = gxpool.tile([128, 2048], BF16, tag="gx")
            nc.gpsimd.dma_gather(
                out_ap=gxt.rearrange("p (o f) -> p o f", o=1),
                in_ap=P_dram[:],
                idxs_ap=idx_sb[:, g * 8:(g + 1) * 8],
                num_idxs=128,
                num_idxs_reg=n128,
                elem_size=G,
                queue_num=g % 4,
            )
            gx_tiles[g] = gxt

        for g in range(N_GATHER):
            emit_gather(g)

        # deferred phase-3 constant DMAs: they arrive during the recurrence
        for k in range(4):
            nc.sync.dma_start(out=whmT[:, k, :], in_=whmT_d.ap()[k])
            nc.sync.dma_start(out=wlogT[:, k, :], in_=wlogT_d.ap()[k])
        for k in range(8):
            nc.sync.dma_start(out=womT[:, k, :], in_=womT_d.ap()[k])
        nc.sync.dma_start(out=blog, in_=blog_d.ap())
        for b in range(BLOC):
            for k in range(2):
                nc.sync.dma_start(out=cnn_sb[:, b, k, :], in_=cnn_d.ap()[b, k])
            for k in range(4):
                nc.sync.dma_start(out=cnnT_sb[:, b, k, :], in_=cnnT_d.ap()[b, k])

        # h state lives TRANSPOSED (h-major) as [128, 2(A/B), 2(sub), 64]:
        # h-chunk k is h2T[:, k % 2, k // 2, :].  c stays batch-folded f32.
        h2T = statepool.tile([128, 2, 2, 64], BF16, tag="hT")
        nc.vector.memset(h2T, 0.0)
        cF = statepool.tile([128, 256], F32, tag="cF")
        nc.vector.memset(cF, 0.0)

        def emit_folds(t):
            """Fold the gathered x-projection for step t into fresh gate
            banks (start=True).  No hT dependency, so these run during the
            previous step's tail."""
            po = (t % 2) * 64
            gx = gx_tiles[t // 2]
            idg = ident[po:po + 64, po:po + 64]
            gps = []
            for n in range(4):
                gp = gpsum.tile([128, 256], F32, tag=f"g{n}")
                c0 = n * 512
                nc.tensor.matmul(gp[0:64, :], idg,
                                 gx[po:po + 64, c0:c0 + 256],
                                 start=True, stop=False, tile_position=(po, 0))
                nc.tensor.matmul(gp[64:128, :], idg,
                                 gx[po:po + 64, c0 + 256:c0 + 512],
                                 start=True, stop=False, tile_position=(po, 64))
                gps.append(gp)
            return gps

        # Per-gate W_hh pair order: gates complete early->late as
        # [i, g, f, o] (for the eager ACT chain), while k in {0,2} pairs
        # lead so the A-half of the previous step's h2T unblocks the burst
        # ~0.4us before the B-half is ready.
        PAIR_ORDER = [(0, 0), (0, 2), (1, 0), (1, 2), (0, 1), (0, 3),
                      (1, 1), (1, 3), (2, 0), (2, 2), (3, 0), (3, 2),
                      (2, 1), (2, 3), (3, 1), (3, 3)]

        junk2 = trpsum.tile([128, 256], F32, tag="junk2")
        gps = emit_folds(0)
        for t in range(T):
            # Gate order along G: [i | g | f | o] (see perm in _prep_inputs).
            # Eager per-gate ACT lets the c-path overlap the f/o MM groups.
            sig_i = tanh_g = sig_o = m1 = None
            kcnt = [0, 0, 0, 0]
            for n, k in PAIR_ORDER:
                gp = gps[n]
                c0 = n * 512
                kcnt[n] += 1
                st = (kcnt[n] == 4)
                hk = h2T[:, k % 2, k // 2, :]
                nc.tensor.matmul(gp[0:64, :], hk,
                                 whhT[:, k, c0:c0 + 256],
                                 start=False, stop=st, tile_position=(0, 0))
                nc.tensor.matmul(gp[64:128, :], hk,
                                 whhT[:, k, c0 + 256:c0 + 512],
                                 start=False, stop=st, tile_position=(0, 64))
                if not st:
                    continue
                if n == 0:
                    sig_i = workpool.tile([128, 256], BF16, tag="sig_i")
                    nc.scalar.activation(sig_i, gps[0], SIG)
                elif n == 1:
                    tanh_g = workpool.tile([128, 256], BF16, tag="tanh_g")
                    nc.scalar.activation(tanh_g, gps[1], TANH)
                    m1 = workpool.tile([128, 256], F32, tag="m1")
                    nc.vector.tensor_mul(m1, sig_i, tanh_g)
                elif n == 2:
                    sig_f = workpool.tile([128, 256], BF16, tag="sig_f")
                    nc.scalar.activation(sig_f, gps[2], SIG)
                    m2 = workpool.tile([128, 256], F32, tag="m2")
                    nc.vector.tensor_mul(m2, sig_f, cF)
                else:
                    sig_o = workpool.tile([128, 256], BF16, tag="sig_o")
                    nc.scalar.activation(sig_o, gps[3], SIG)
            # next step's folds go ahead of the tail transposes in the PE
            # FIFO so the PE stays busy while the c-path chain completes
            if t + 1 < T:
                next_gps = emit_folds(t + 1)
            # --- A/B-split tail: each half of the c/h chain unblocks half
            # of the next step's W_hh pairs ---
            cF2 = statepool.tile([128, 256], F32, tag="cF")
            nc.vector.tensor_add(cF2[:, 0:128], m1[:, 0:128], m2[:, 0:128])
            nc.vector.tensor_add(cF2[:, 128:256], m1[:, 128:256],
                                 m2[:, 128:256])
            cF = cF2
            tc2 = workpool.tile([128, 256], BF16, tag="tc2")
            nc.scalar.activation(tc2[:, 0:128], cF[:, 0:128], TANH)
            nc.scalar.activation(tc2[:, 128:256], cF[:, 128:256], TANH)
            # transpose sig(o) and tanh(c2) via REGULAR identity matmuls
            # (counts as PE-busy for HAM, exact, out=f32 psum):
            #   xT[:, s*64:(s+1)*64] = x-chunk (2s + a) in h-major form
            soT = trpsum.tile([128, 2, 128], F32, tag="soT")
            nc.tensor.matmul(soT[:, 0, :], sig_o[:, 0:128], ident,
                             start=True, stop=True)
            nc.tensor.matmul(soT[:, 1, :], sig_o[:, 128:256], ident,
                             start=True, stop=True)
            # junk matmul hooked on sig(o): pads the PE gap while tanh runs
            nc.tensor.matmul(junk2[0:64, :], sig_o[:, 0:64], junkin[:, 0:256],
                             start=True, stop=True)
            soT_sb = workpool.tile([128, 256], BF16, tag="soT_sb")
            nc.vector.tensor_copy(soT_sb[:, 0:128], soT[:, 0, :])
            nc.vector.tensor_copy(soT_sb[:, 128:256], soT[:, 1, :])
            tcTA = trpsum.tile([128, 128], F32, tag="tcTA")
            nc.tensor.matmul(tcTA, tc2[:, 0:128], ident,
                             start=True, stop=True)
            # junk matmul hooked on tanh(c2) A-half: pads until mul-A lands
            nc.tensor.matmul(junk2[0:64, :], tc2[:, 0:64], junkin[:, 0:256],
                             start=True, stop=True)
            tcTB = trpsum.tile([128, 128], F32, tag="tcTB")
            nc.tensor.matmul(tcTB, tc2[:, 128:256], ident,
                             start=True, stop=True)
            h2T = statepool.tile([128, 2, 2, 64], BF16, tag="hT")
            nc.vector.tensor_mul(h2T[:, 0, :, :].rearrange("p s b -> p (s b)"),
                                 soT_sb[:, 0:128], tcTA)
            nc.vector.tensor_mul(h2T[:, 1, :, :].rearrange("p s b -> p (s b)"),
                                 soT_sb[:, 128:256], tcTB)
            # stash this core's 8 rows for phase 3 (b-major cols b*T + t):
            # h-chunks {0,2} come from A (a=0), {1,3} from B (a=1)
            nc.vector.tensor_copy(HT[:, 0:4:2, t:BLOC * T:T],
                                  h2T[:, 0, :, 0:BLOC])
            nc.scalar.copy(HT[:, 1:4:2, t:BLOC * T:T],
                           h2T[:, 1, :, 0:BLOC])
            gps = next_gps if t + 1 < T else None
        # keep the junk bank live so the pad matmuls aren't dead code
        jout2 = workpool.tile([1, 8], F32, tag="jout2")
        nc.vector.tensor_copy(jout2, junk2[0:1, 0:8])
        nc.sync.dma_start(out=junk_dram[:], in_=jout2)

    # ---- phase 3: attention + logits for this core's 8 batch rows -----
    NB = BLOC * T  # 1200 columns, ordered b*T + t
    p3pool = tc.alloc_tile_pool(name="p3", bufs=1)
    mappedT = p3pool.tile([128, 4, NB], BF16)

    # stage A: mappedT[h', bt] = sum_h W_hm[h', h] * hT[h, bt]
    with tc.tile_pool(name="bigpA", bufs=1, space="PSUM") as bigpsum, \
         tc.tile_pool(name="p3m", bufs=3) as p3m:
        NCH = [(j * 300, 300) for j in range(4)]
        for m in range(4):
            ps = bigpsum.tile([128, 4, 512], F32, tag="big")
            for k in range(4):
                for j, (n0, nsz) in enumerate(NCH):
                    nc.tensor.matmul(
                        ps[:, j, :nsz], whmT[:, k, m * 128:(m + 1) * 128],
                        HT[:, k, n0:n0 + nsz],
                        start=(k == 0), stop=(k == 3))
            for j, (n0, nsz) in enumerate(NCH):
                _copy(nc, j, mappedT[:, m, n0:n0 + nsz], ps[:, j, :nsz])

    ctxT = p3pool.tile([128, 4, NB], BF16)
    outT = p3pool.tile([128, 4, NB], BF16)

    # stage B: attention + context (staged; apsum/wpsum double-buffered)
    with tc.tile_pool(name="p3w", bufs=4) as p3work, \
         tc.tile_pool(name="bigp", bufs=1, space="PSUM") as bigpsum, \
         tc.tile_pool(name="apsum", bufs=2, space="PSUM") as apsum, \
         tc.tile_pool(name="wpsum", bufs=2, space="PSUM") as wpsum:
        for b in range(BLOC):
            wT = p3work.tile([128, 2, T], BF16, tag="wT")
            for th in range(2):
                t0 = th * 75
                pa = apsum.tile([75, 256], F32, tag="attn")
                for k in range(4):
                    nc.tensor.matmul(
                        pa, mappedT[:, k, b * T + t0:b * T + t0 + 75],
                        cnnT_sb[:, b, k, :], start=(k == 0), stop=(k == 3))
                amax = p3work.tile([75, 1], F32, tag="amax")
                nc.vector.tensor_reduce(amax, pa, X, MAXOP, negate=True)
                w_sb = p3work.tile([75, 256], BF16, tag="w_sb")
                sumex = p3work.tile([75, 1], F32, tag="sumex")
                nc.scalar.activation(w_sb, pa, EXP, bias=amax,
                                     accum_out=sumex)
                rs = p3work.tile([75, 1], F32, tag="rs")
                nc.vector.reciprocal(rs, sumex)
                nc.vector.tensor_scalar_mul(w_sb, w_sb, rs)
                pw = wpsum.tile([128, 2, 76], BF16, tag="wtp")
                for k in range(2):
                    nc.tensor.transpose(
                        pw[:, k, :75], w_sb[:, k * 128:(k + 1) * 128],
                        ident[:75, :75])
                nc.scalar.copy(wT[:, :, t0:t0 + 75], pw[:, :, :75])
            # ctxT[h, t] = sum_l cnn[b, l, h] * w[t, l]
            pc = bigpsum.tile([128, 4, 512], F32, tag="big")
            for k in range(2):
                for m in range(4):
                    nc.tensor.matmul(
                        pc[:, m, :T], cnn_sb[:, b, k, m * 128:(m + 1) * 128],
                        wT[:, k, :], start=(k == 0), stop=(k == 1))
            for m in range(4):
                _copy(nc, m, ctxT[:, m, b * T:(b + 1) * T], pc[:, m, :T])

        # outT[h', bt] = sum_k W_om.T[k, h'] * [ctxT; HT][k, bt]
        for m in range(4):
            ps = bigpsum.tile([128, 4, 512], F32, tag="big")
            for k in range(8):
                src = ctxT if k < 4 else HT
                kk = k % 4
                for j, (n0, nsz) in enumerate(NCH):
                    nc.tensor.matmul(
                        ps[:, j, :nsz], womT[:, k, m * 128:(m + 1) * 128],
                        src[:, kk, n0:n0 + nsz],
                        start=(k == 0), stop=(k == 7))
            for j, (n0, nsz) in enumerate(NCH):
                _copy(nc, j, outT[:, m, n0:n0 + nsz], ps[:, j, :nsz])

    # stage C: logits + log_softmax (lpsum triple-buffered so the next
    # group's matmuls overlap this group's softmax chain)
    with tc.tile_pool(name="p3l", bufs=4) as p3lw, \
         tc.tile_pool(name="lpsum", bufs=3, space="PSUM") as lpsum, \
         tc.tile_pool(name="outp", bufs=3) as outpool:
        for b in range(BLOC):
            for th in range(2):
                t0 = th * 75
                pl = lpsum.tile([75, 1024], F32, tag="logit")
                for k in range(4):
                    for v0, vs in ((0, 512), (512, 488)):
                        nc.tensor.matmul(
                            pl[:, v0:v0 + vs],
                            outT[:, k, b * T + t0:b * T + t0 + 75],
                            wlogT[:, k, v0:v0 + vs],
                            start=(k == 0), stop=False)
                for v0, vs in ((0, 512), (512, 488)):
                    nc.tensor.matmul(
                        pl[:, v0:v0 + vs], ones2[0:1, :75], blog[:, v0:v0 + vs],
                        start=False, stop=True)
                ex = p3lw.tile([75, V], BF16, tag="ex")
                sume = p3lw.tile([75, 1], F32, tag="sume")
                nc.scalar.activation(ex[:, :V], pl[:, :V], EXP,
                                     accum_out=sume)
                lns = p3lw.tile([75, 1], F32, tag="lns")
                nc.scalar.activation(lns, sume, LN)
                outt = outpool.tile([75, V], F32, tag="outt")
                nc.vector.tensor_scalar(
                    out=outt[:, :512], in0=pl[:, :512], scalar1=lns,
                    scalar2=None, op0=SUB)
                nc.vector.tensor_scalar(
                    out=outt[:, 512:V], in0=pl[:, 512:V], scalar1=lns,
                    scalar2=None, op0=SUB)
                nc.sync.dma_start(out=out_d.ap()[b, t0:t0 + 75, :], in_=outt)

    p3pool.release()
    singles.release()


def _prep_inputs(cnn_feats, seq, embed_table, W_ih, b_ih, W_hh, b_hh,
                 W_hm, W_om, W_logit, b_logit):
    bf = NPBF16
    perm = np.r_[0:512, 1024:1536, 512:1024, 1536:2048]  # [i|g|f|o]

    def pack_T(w):  # [out, in] -> [in//128, 128, out] transposed chunks
        wt = np.ascontiguousarray(w.T.astype(bf))
        return wt.reshape(-1, 128, w.shape[0])

    common = {
        "whhT": pack_T(np.asarray(W_hh)[perm]),
        "wihT": pack_T(np.asarray(W_ih)[perm]),
        "embT": pack_T(np.asarray(embed_table)),
        "bias2": np.stack([np.asarray(b_ih)[perm],
                           np.asarray(b_hh)[perm]]).astype(bf),
        "whmT": pack_T(np.asarray(W_hm)),
        "womT": pack_T(np.asarray(W_om)),
        "wlogT": pack_T(np.asarray(W_logit)),
        "blog": np.asarray(b_logit).astype(bf).reshape(1, V),
    }
    cnn = np.asarray(cnn_feats).astype(bf)            # [64, 256, 512]
    cnnK = cnn.reshape(B, 2, 128, H)
    cnnT = np.ascontiguousarray(cnn.transpose(0, 2, 1)).reshape(B, 4, 128, L)
    seq_np = np.asarray(seq).astype(np.int64)         # [64, 150]

    in_maps = []
    for k in range(NCORES):
        order = (np.arange(B) + BLOC * k) % B
        flat = seq_np[order].T.reshape(-1)            # t-major [9600]
        wrapped = np.zeros((128, T * B // 16), np.int16)
        wrapped[:16, :] = flat.reshape(-1, 16).T
        m = dict(common)
        m["cnn"] = np.ascontiguousarray(cnnK[BLOC * k:BLOC * (k + 1)])
        m["cnnT"] = np.ascontiguousarray(cnnT[BLOC * k:BLOC * (k + 1)])
        m["idx"] = wrapped
        in_maps.append(m)
    return in_maps


_CACHE = {}


def kernel(**inputs):
    in_maps = _prep_inputs(**inputs)
    if "nc" not in _CACHE:
        _CACHE["nc"] = build_program()
    nc = _CACHE["nc"]
    trace = bool(int(os.environ.get("KERNEL_TRACE", "0")))
    res = run_bass_kernel_spmd(
        nc, in_maps, core_ids=list(range(NCORES)), trace=trace)
    if trace:
        print(f"HW exec time: {res.exec_time_ns} ns")
    out = np.concatenate([r["out"] for r in res.results], axis=0)  # [64,150,1000]
    return out.astype(np.float32)


if __name__ == "__main__":
    sys.path.insert(0, "/root/problem")
    import reference
    inputs = {k: np.asarray(v) for k, v in reference.setup_inputs().items()}
    got = kernel(**inputs)
    print("out shape", got.shape)
